# revision 5
# baseline (speedup 1.0000x reference)
"""DCell grouped Linear + tanh + BatchNorm1d kernel for Trainium2 (8 NeuronCores).

Problem: S=2048 independent subsystems, each computing
    h = tanh(x[B,I] @ W[O,I]^T + b);  y = BN_batch(h) * gamma + beta, masked.
Sharding: subsystem dim split across 8 cores (256 subsystems/core), no
cross-core communication.

Per-core kernel layout (per block of 16 subsystems):
  - PSUM tile [80, 16*32]: one bank; bias added via a K=16 matmul of the
    stacked bias block against a constant block-identity, then 2 accumulating
    K=128 matmuls per subsystem (W stationary, x moving).
  - tanh on ScalarE (one table set for the whole kernel; Identity/Square share
    it, sqrt is never used -> no table thrash).
  - bn_stats on VectorE gives per-subsystem even/odd mean & SS over the batch;
    combined per 4-block group into mean/var, rsqrt via 2 Newton iterations
    (magic-constant seed) entirely on VectorE.
  - final y = t*scale + shift per subsystem, split between VectorE
    (tensor_scalar) and ScalarE (Identity activation with per-partition
    scale/bias APs) to balance engine load.
Host side pre-transposes inputs so every DMA row is >=512B contiguous.
"""

import sys

sys.path.insert(0, "/opt/trn_rl_repo")

import dataclasses
import numpy as np

from concourse import bass, tile
from concourse.bass_utils import run_bass_kernel_spmd
import concourse.mybir as mybir

F32 = mybir.dt.float32
I32 = mybir.dt.int32
ALU = mybir.AluOpType
AF = mybir.ActivationFunctionType

S, B, I, O = 2048, 32, 256, 80
NCORES = 8
SC = S // NCORES  # 256 subsystems per core
BLK = 16          # subsystems per PSUM block
GRP = 4           # blocks per stats group
EPS = 1e-5
RSQRT_MAGIC = 0x5F3759DF


def split_multiwaits(nc, maxw=1):
    """walrus in this container rejects instructions with >maxw sem waits;
    move excess waits onto preceding same-engine Drain carriers."""
    for f in nc.m.functions:
        for blk in f.blocks:
            insts = blk.instructions
            if not any(
                getattr(i, "sync_info", None)
                and i.sync_info.on_wait
                and len(i.sync_info.on_wait) > maxw
                for i in insts
            ):
                continue
            new_insts = []
            for ins in insts:
                si = getattr(ins, "sync_info", None)
                if si and si.on_wait and len(si.on_wait) > maxw:
                    waits = list(si.on_wait)
                    k = 0
                    while len(waits) > maxw:
                        chunk, waits = waits[:maxw], waits[maxw:]
                        new_insts.append(
                            mybir.InstDrain(
                                name=f"{ins.name}-ws{k}",
                                opcode="Drain",
                                engine=ins.engine,
                                debug=ins.debug,
                                ins=[],
                                outs=[],
                                sync_info=mybir.SyncInfo(on_wait=chunk, on_update=[]),
                            )
                        )
                        k += 1
                    new_insts.append(
                        dataclasses.replace(
                            ins,
                            sync_info=mybir.SyncInfo(
                                on_wait=waits, on_update=list(si.on_update or [])
                            ),
                        )
                    )
                else:
                    new_insts.append(ins)
            blk.instructions = new_insts


def build_nc(sc=SC):
    nblk = sc // BLK
    ngrp = (nblk + GRP - 1) // GRP

    nc = bass.Bass("TRN2", target_bir_lowering=False, debug=False, num_devices=1)

    xt = nc.dram_tensor("xt", [2, 128, sc * B], F32, kind="ExternalInput")
    wt = nc.dram_tensor("wt", [2, 128, sc * O], F32, kind="ExternalInput")
    bt = nc.dram_tensor("bt", [BLK, nblk * O], F32, kind="ExternalInput")
    gt = nc.dram_tensor("gt", [O, sc], F32, kind="ExternalInput")
    bet = nc.dram_tensor("bet", [O, sc], F32, kind="ExternalInput")
    ident = nc.dram_tensor("ident", [BLK, BLK * B], F32, kind="ExternalInput")
    yo = nc.dram_tensor("yo", [O, sc, B], F32, kind="ExternalOutput")

    with tile.TileContext(nc) as tc:
        with (
            tc.tile_pool(name="const", bufs=1) as cpool,
            tc.tile_pool(name="w", bufs=2) as wpool,
            tc.tile_pool(name="x", bufs=2) as xpool,
            tc.tile_pool(name="t", bufs=GRP + 2) as tpool,
            tc.tile_pool(name="y", bufs=3) as ypool,
            tc.tile_pool(name="gstat", bufs=2) as gpool,
            tc.tile_pool(name="chain", bufs=2) as spool,
            tc.tile_pool(name="psum", bufs=4, space="PSUM") as ppool,
        ):
            bt_t = cpool.tile([BLK, nblk * O], F32)
            nc.sync.dma_start(bt_t[:], bt[:])
            gt_t = cpool.tile([O, sc], F32)
            nc.sync.dma_start(gt_t[:], gt[:])
            bet_t = cpool.tile([O, sc], F32)
            nc.sync.dma_start(bet_t[:], bet[:])
            id_t = cpool.tile([BLK, BLK * B], F32)
            nc.sync.dma_start(id_t[:], ident[:])
            k_t = cpool.tile([O, GRP * BLK], I32)
            nc.vector.memset(k_t[:], RSQRT_MAGIC)

            for g in range(ngrp):
                blocks = range(g * GRP, min((g + 1) * GRP, nblk))
                nb = len(blocks)
                gw = nb * BLK  # subsystems in this group
                sums_g = gpool.tile([O, GRP * BLK], F32, tag="sums")
                ssq_g = gpool.tile([O, GRP * BLK], F32, tag="ssq")
                t_tiles = {}
                for bi, blk in enumerate(blocks):
                    w_t = wpool.tile([128, 2, BLK * O], F32, tag="w")
                    nc.sync.dma_start(
                        w_t[:, 0, :], wt[0, :, blk * BLK * O : (blk + 1) * BLK * O]
                    )
                    nc.sync.dma_start(
                        w_t[:, 1, :], wt[1, :, blk * BLK * O : (blk + 1) * BLK * O]
                    )
                    x_t = xpool.tile([128, 2, BLK * B], F32, tag="x")
                    nc.sync.dma_start(
                        x_t[:, 0, :], xt[0, :, blk * BLK * B : (blk + 1) * BLK * B]
                    )
                    nc.sync.dma_start(
                        x_t[:, 1, :], xt[1, :, blk * BLK * B : (blk + 1) * BLK * B]
                    )

                    h = ppool.tile([O, BLK, B], F32, tag="h")
                    # bias: h[o, j*32+c] = b_blk[j, o]; lhsT=[16,80] bias rows,
                    # rhs = block-identity [16, 512]
                    nc.tensor.matmul(
                        h[:, :, :],
                        bt_t[:, blk * O : (blk + 1) * O],
                        id_t[:, :],
                        start=True,
                        stop=False,
                    )
                    for j in range(BLK):
                        for k in range(2):
                            nc.tensor.matmul(
                                h[:, j, :],
                                w_t[:, k, j * O : (j + 1) * O],
                                x_t[:, k, j * B : (j + 1) * B],
                                start=False,
                                stop=(j == BLK - 1 and k == 1),
                            )

                    t_t = tpool.tile([O, BLK, B], F32, tag="t")
                    nc.scalar.activation(t_t[:, :, :], h[:, :, :], AF.Tanh)
                    t_tiles[blk] = t_t

                    nc.vector.tensor_reduce(
                        sums_g[:, bi * BLK : (bi + 1) * BLK],
                        t_t[:, :, :],
                        axis=mybir.AxisListType.X,
                        op=ALU.add,
                    )
                    sq_t = tpool.tile([O, BLK, B], F32, tag="sq")
                    nc.scalar.square(sq_t[:, :, :], t_t[:, :, :])
                    nc.vector.tensor_reduce(
                        ssq_g[:, bi * BLK : (bi + 1) * BLK],
                        sq_t[:, :, :],
                        axis=mybir.AxisListType.X,
                        op=ALU.add,
                    )

                # --- group stats chain on [O, gw] tiles ---
                mean = spool.tile([O, GRP * BLK], F32, tag="mean")
                nc.vector.tensor_scalar(
                    mean[:, :gw], sums_g[:, :gw], 1.0 / B, None, ALU.mult
                )
                em2e = spool.tile([O, GRP * BLK], F32, tag="em2e")
                nc.vector.tensor_scalar(
                    em2e[:, :gw], ssq_g[:, :gw], 1.0 / B, EPS, ALU.mult, ALU.add
                )
                m2 = spool.tile([O, GRP * BLK], F32, tag="m2")
                nc.vector.tensor_mul(m2[:, :gw], mean[:, :gw], mean[:, :gw])
                veps = spool.tile([O, GRP * BLK], F32, tag="veps")
                nc.vector.tensor_tensor(
                    veps[:, :gw], em2e[:, :gw], m2[:, :gw], ALU.subtract
                )

                # rsqrt(veps) via magic seed + 2 Newton iterations
                sh = spool.tile([O, GRP * BLK], I32, tag="sh")
                nc.vector.tensor_scalar(
                    sh[:, :gw],
                    veps[:, :gw].bitcast(I32),
                    1,
                    None,
                    ALU.logical_shift_right,
                )
                y0 = spool.tile([O, GRP * BLK], F32, tag="y0")
                nc.vector.tensor_tensor(
                    y0[:, :gw].bitcast(I32), k_t[:, :gw], sh[:, :gw], ALU.subtract
                )
                rs = y0
                for it in range(2):
                    a = spool.tile([O, GRP * BLK], F32, tag=f"nra{it}")
                    nc.vector.tensor_mul(a[:, :gw], rs[:, :gw], rs[:, :gw])
                    bq = spool.tile([O, GRP * BLK], F32, tag=f"nrb{it}")
                    nc.vector.tensor_mul(bq[:, :gw], a[:, :gw], veps[:, :gw])
                    cf = spool.tile([O, GRP * BLK], F32, tag=f"nrc{it}")
                    nc.vector.tensor_scalar(
                        cf[:, :gw], bq[:, :gw], -0.5, 1.5, ALU.mult, ALU.add
                    )
                    yn = spool.tile([O, GRP * BLK], F32, tag=f"nry{it}")
                    nc.vector.tensor_mul(yn[:, :gw], rs[:, :gw], cf[:, :gw])
                    rs = yn

                g0 = g * GRP * BLK
                s2 = spool.tile([O, GRP * BLK], F32, tag="s2")
                nc.vector.tensor_mul(s2[:, :gw], rs[:, :gw], gt_t[:, g0 : g0 + gw])
                mc = spool.tile([O, GRP * BLK], F32, tag="mc")
                nc.vector.tensor_mul(mc[:, :gw], mean[:, :gw], s2[:, :gw])
                cc = spool.tile([O, GRP * BLK], F32, tag="cc")
                nc.vector.tensor_tensor(
                    cc[:, :gw], bet_t[:, g0 : g0 + gw], mc[:, :gw], ALU.subtract
                )

                # --- apply y = t*s2 + cc, split DVE/ACT, and store ---
                for bi, blk in enumerate(blocks):
                    t_t = t_tiles[blk]
                    y_t = ypool.tile([O, BLK, B], F32, tag="y")
                    for j in range(BLK):
                        lj = bi * BLK + j
                        if j % 8 < 3:  # 3/8 of applies on ScalarE
                            nc.scalar.activation(
                                y_t[:, j, :],
                                t_t[:, j, :],
                                AF.Identity,
                                bias=cc[:, lj : lj + 1],
                                scale=s2[:, lj : lj + 1],
                            )
                        else:
                            nc.vector.tensor_scalar(
                                y_t[:, j, :],
                                t_t[:, j, :],
                                s2[:, lj : lj + 1],
                                cc[:, lj : lj + 1],
                                ALU.mult,
                                ALU.add,
                            )
                    nc.sync.dma_start(
                        yo[:, blk * BLK : (blk + 1) * BLK, :], y_t[:, :, :]
                    )

    return nc


_NC_CACHE = {}


def _get_nc(sc=SC):
    if sc not in _NC_CACHE:
        nc = build_nc(sc)
        split_multiwaits(nc)  # walrus compat; breaks CoreSim, HW-path only
        _NC_CACHE[sc] = nc
    return _NC_CACHE[sc]


def prep_core_inputs(xm, W, b, gm, bem, s0, s1):
    """Build one core's input map from full pre-masked arrays."""
    sc = s1 - s0
    nblk = sc // BLK
    xs = xm[s0:s1]  # [sc, B, I]
    ws = W[s0:s1]  # [sc, O, I]
    xt = np.ascontiguousarray(xs.transpose(2, 0, 1)).reshape(2, 128, sc * B)
    wt = np.ascontiguousarray(ws.transpose(2, 0, 1)).reshape(2, 128, sc * O)
    bt = np.ascontiguousarray(
        b[s0:s1].reshape(nblk, BLK, O).transpose(1, 0, 2)
    ).reshape(BLK, nblk * O)
    gt = np.ascontiguousarray(gm[s0:s1].T)
    bet = np.ascontiguousarray(bem[s0:s1].T)
    ident = np.zeros((BLK, BLK * B), np.float32)
    for j in range(BLK):
        ident[j, j * B : (j + 1) * B] = 1.0
    return {
        "xt": xt,
        "wt": wt,
        "bt": bt,
        "gt": gt,
        "bet": bet,
        "ident": ident,
    }


def kernel(x, W, b, gamma, beta, in_mask, out_mask):
    x = np.asarray(x, np.float32)
    W = np.asarray(W, np.float32)
    b = np.asarray(b, np.float32)
    gamma = np.asarray(gamma, np.float32)
    beta = np.asarray(beta, np.float32)
    in_mask = np.asarray(in_mask, np.float32)
    out_mask = np.asarray(out_mask, np.float32)

    xm = x * in_mask[:, None, :]
    gm = gamma * out_mask
    bem = beta * out_mask

    in_maps = [
        prep_core_inputs(xm, W, b, gm, bem, c * SC, (c + 1) * SC)
        for c in range(NCORES)
    ]
    nc = _get_nc()
    res = run_bass_kernel_spmd(nc, in_maps, core_ids=list(range(NCORES)))

    out = np.empty((S, B, O), np.float32)
    for c in range(NCORES):
        yo = res.results[c]["yo"]  # [O, SC, B]
        out[c * SC : (c + 1) * SC] = yo.transpose(1, 2, 0)
    return out
